# revision 77
# baseline (speedup 1.0000x reference)
"""Cross-attention kernel for Trainium2 (8 NeuronCores, SPMD).

Problem: B=4, Nq=1024, Nk=2048, D=512, 8 heads x 64 head-dim, fp32,
full-tensor bias added to scores before softmax.

Sharding: (batch, head-half) -> 8 shards. Core 2b+hh computes heads
[4hh, 4hh+4) of batch b over ALL 1024 queries. The output projection is
computed per head-half (row-sharded Wo) and the two partial [512,1024]
results of a batch are summed on the host (plus bo).

Device layout: attention tensors transposed (feature/key dim on
partitions) so every contraction lands on the partition axis:
  QT[d', q] = (SCALE*Wq_hh) @ xT       KT[d', k] = Wk_hh @ ctxT
  V[k, i]   = ctxT.T @ Wv_hh.T
  ST[k, q]  = KT_h.T @ QT_h            (K=64; head pair in PE row groups)
  E  = exp(ST)                         (ACT, FD=1024 per instruction --
                                        the pacer: 64 x ~1.15us)
  et = E * exp(biasT - 4)              (DVE fp16 2x; -4 cancels in the
                                        softmax normalization)
  o2[i(+1), q] = [V_h | 1].T @ et      (ones column gives row sums)
  OT = o2 * recip(sum)                 (sums DMA-reshaped to [128,8] for
                                        parallel reciprocal, DMA-replicated
                                        back; multiply reads o2 PSUM direct)
  yT[d, q] = Wo_hh @ OT                (partial; host sums the halves)

Changes vs the 150us baseline:
  - startup: input DMAs ordered by criticality over the 3 DMA queues
    (sync/scalar/gpsimd): wk+ctx[keys 0:512] -> wq+x[sect0] -> wv ->
    bias chunks, each queue carrying a slice of every wave; the exp
    table is primed at t=0 by a dummy activation.  ctx/x are shipped as
    separate DRAM tensors per column block so every transfer is linear.
  - the exp stream starts as soon as chunk 0 of (sect0, pair0) is
    projected; the bulk of ctx/x/bias streams in underneath section 0.
  - score matmul pair emitted adjacently with explicit tile_position
    (distinct PE row groups).
  - fillers rebalanced: K/Q projections ride as 2-matmul groups in a
    psA slot, spread to the latest chunks their consumers allow;
    V lookahead in si=0; section-0 out-proj late in si=3 so the si=2
    norm's reciprocal round-trip stays off the PE queue's critical path.
  - mid-kernel norms round-trip sums through DRAM for the [128,8]
    parallel reciprocal (latency hidden by the exp stream); the LAST
    norm instead uses ACT Ln/Exp(-x) (ACT idle after the final exp) and
    K=1 matmul broadcasts, avoiding exposed DMA latency in the tail.
Queries processed in two 512-column sections; each o2 accumulator is one
PSUM bank.  Matmul operands fp16 (fp32 PSUM accumulate); fp8 was tested
and rejected: softmax concentration exposes per-element quantization
(9.5% max err for fp8 Q/K, 3.3% for fp8 V - over the 2e-2 gate).
"""

import numpy as np
import concourse.bass as bass
import concourse.bacc as bacc
import concourse.mybir as mybir
import concourse.tile as tile
from concourse import bass_utils

HEADS = 8
DH = 64
D = 512
NH = 4            # heads per core
INNER = NH * DH   # 256
NQ = 1024         # all queries on every core
NS = 512          # query section width
NK = 2048
KC = NK // 128    # 16 key chunks
SCALE = DH ** -0.5
BSHIFT = 4.0      # exp(bias - BSHIFT): keeps fp16 weights in range

F32 = mybir.dt.float32
F16 = mybir.dt.float16
AF = mybir.ActivationFunctionType


def _bcast2(ap, n):
    """[128, F] -> [128, n, F] with a step-0 middle dim."""
    return bass.AP(ap.tensor, ap.offset, [ap.ap[0], [0, n], ap.ap[1]])


def _build_nc():
    nc = bacc.Bacc("TRN2", target_bir_lowering=False, debug=False)

    xT_d = [nc.dram_tensor(f"xT{s}", [D, NS], F16, kind="ExternalInput")
            for s in range(2)]
    ctxT0_d = nc.dram_tensor("ctxT0", [D, 512], F16, kind="ExternalInput")
    ctxTr_d = nc.dram_tensor("ctxTr", [D, 1536], F16, kind="ExternalInput")
    expB_d = [nc.dram_tensor(f"expB{s}", [NK, NS], F16, kind="ExternalInput")
              for s in range(2)]
    wqT_d = nc.dram_tensor("wqT", [D, INNER], F16, kind="ExternalInput")
    wkT_d = nc.dram_tensor("wkT", [D, INNER], F16, kind="ExternalInput")
    wvT_d = nc.dram_tensor("wvT", [D, INNER], F16, kind="ExternalInput")
    woT_d = nc.dram_tensor("woT", [INNER, D], F16, kind="ExternalInput")
    yT_d = nc.dram_tensor("yT", [D, NQ], F16, kind="ExternalOutput")
    scr_d = [nc.dram_tensor(f"scr{i}", [2 * NS], F16) for i in range(4)]

    with tile.TileContext(nc) as tc, nc.allow_low_precision(
            reason="fp16 matmul operands, fp32 accumulation"):
        with (
            tc.tile_pool(name="const", bufs=1) as const,
            tc.tile_pool(name="main", bufs=1) as main,
            tc.tile_pool(name="work", bufs=8) as work,
            tc.tile_pool(name="nrp", bufs=2) as nrp,
            tc.tile_pool(name="psS", bufs=2, space="PSUM") as psS,
            tc.tile_pool(name="psO", bufs=2, space="PSUM") as psO,
            tc.tile_pool(name="psA", bufs=2, space="PSUM") as psA,
        ):
            # ---- ACT table prime: dummy exp with no deps runs at t=0 so
            # the one-and-only table load overlaps the input DMA ----
            dume = const.tile([1, 8], F16, name="dume", tag="dume")
            nc.vector.memset(dume, 0.0)
            nc.scalar.activation(dume, dume, AF.Exp)

            # ---- input DMA: whole-tile transfers, interleaved across the
            # four queues by criticality (wk+ctx feed k_proj first, then
            # x/wv, then bias chunk 0-3) ----
            wq = [const.tile([128, INNER], F16, name=f"wq{i}", tag=f"wq{i}") for i in range(4)]
            wk = [const.tile([128, INNER], F16, name=f"wk{i}", tag=f"wk{i}") for i in range(4)]
            wv = [const.tile([128, INNER], F16, name=f"wv{i}", tag=f"wv{i}") for i in range(4)]
            wo = [const.tile([128, D], F16, name=f"wo{i}", tag=f"wo{i}") for i in range(2)]
            xts = [const.tile([128, NQ], F16, name=f"xts{i}", tag=f"xts{i}") for i in range(4)]
            ctx = [const.tile([128, NK], F16, name=f"ctx{i}", tag=f"ctx{i}") for i in range(4)]
            eB = [main.tile([128, NQ], F16, name=f"eB{c}", tag=f"eB{c}")
                  for c in range(KC)]

            # Criticality waves.  scalar queue carries only the earliest
            # pieces (it must go quiet before the exp stream); sync+gpsimd
            # carry the bulk.  Every transfer is a linear DRAM read.
            def ld_eB(q, c, s=0):
                # per-section halves: section 1 of every chunk is consumed
                # a whole section later, so it streams after all of
                # section 0 and never crowds the critical DMA window
                q.dma_start(out=eB[c][:, s * NS:(s + 1) * NS],
                            in_=expB_d[s][c * 128:(c + 1) * 128, :])

            def r128(i):
                return slice(i * 128, (i + 1) * 128)

            # Critical pieces balanced ~7-8 per queue in dependency order:
            # each queue carries a slice of every wave so wk/ctx0 (k00),
            # wq/x0 (q00), wv (v01) and eB0/1 land as early as possible.
            crit = [
                # (queue idx, dst, src)  -- 0=sync 1=scalar 2=gpsimd
                (0, wk[0], wkT_d[r128(0), :]),
                (0, ctx[0][:, 0:512], ctxT0_d[r128(0), :]),
                (1, wk[1], wkT_d[r128(1), :]),
                (1, ctx[1][:, 0:512], ctxT0_d[r128(1), :]),
                (2, wk[2], wkT_d[r128(2), :]),
                (2, ctx[2][:, 0:512], ctxT0_d[r128(2), :]),
                (0, wk[3], wkT_d[r128(3), :]),
                (0, ctx[3][:, 0:512], ctxT0_d[r128(3), :]),
                (1, wq[0], wqT_d[r128(0), :]),
                (1, xts[0][:, 0:NS], xT_d[0][r128(0), :]),
                (2, wq[1], wqT_d[r128(1), :]),
                (2, xts[1][:, 0:NS], xT_d[0][r128(1), :]),
                (0, wq[2], wqT_d[r128(2), :]),
                (0, xts[2][:, 0:NS], xT_d[0][r128(2), :]),
                (1, wq[3], wqT_d[r128(3), :]),
                (1, xts[3][:, 0:NS], xT_d[0][r128(3), :]),
                (2, wv[0], wvT_d[r128(0), :]),
                (2, wv[1], wvT_d[r128(1), :]),
                (0, wv[2], wvT_d[r128(2), :]),
                (0, wv[3], wvT_d[r128(3), :]),
            ]
            qs = [nc.sync, nc.scalar, nc.gpsimd]
            for qi_, dst, src in crit:
                qs[qi_].dma_start(out=dst, in_=src)
            ld_eB(nc.gpsimd, 0)
            ld_eB(nc.gpsimd, 1)
            # bulk (sync/gpsimd only; scalar goes quiet for the exps):
            # ctx nt1-3 first (k0nt1 filler + V lookahead need them),
            # bias chunks interleaved in consumption order, x sect1, wo
            nc.sync.dma_start(out=ctx[0][:, 512:2048], in_=ctxTr_d[r128(0), :])
            nc.gpsimd.dma_start(out=ctx[1][:, 512:2048], in_=ctxTr_d[r128(1), :])
            ld_eB(nc.sync, 2)
            ld_eB(nc.gpsimd, 3)
            nc.sync.dma_start(out=ctx[2][:, 512:2048], in_=ctxTr_d[r128(2), :])
            nc.gpsimd.dma_start(out=ctx[3][:, 512:2048], in_=ctxTr_d[r128(3), :])
            ld_eB(nc.sync, 4)
            ld_eB(nc.gpsimd, 5)
            ld_eB(nc.sync, 6)
            ld_eB(nc.gpsimd, 7)
            for i in range(4):
                q = nc.sync if i % 2 == 0 else nc.gpsimd
                q.dma_start(out=xts[i][:, NS:NQ], in_=xT_d[1][r128(i), :])
            for c in range(8, KC):
                ld_eB(nc.sync if c % 2 == 0 else nc.gpsimd, c)
            # section-1 bias halves: consumed from si=1 onward
            for c in range(KC):
                ld_eB(nc.sync if c % 2 == 0 else nc.gpsimd, c, 1)
            nc.sync.dma_start(out=wo[0], in_=woT_d[r128(0), :])
            nc.gpsimd.dma_start(out=wo[1], in_=woT_d[r128(1), :])

            KT = [main.tile([128, NK], F16, name=f"KT{p}", tag=f"KT{p}") for p in range(2)]
            QT = [main.tile([128, NQ], F16, name=f"QT{p}", tag=f"QT{p}") for p in range(2)]
            OT = [main.tile([128, NQ], F16, name=f"OT{p}", tag=f"OT{p}") for p in range(2)]
            Vo = [main.tile([128, NH, DH + 1], F16, name=f"Vo{c}", tag=f"Vo{c}")
                  for c in range(KC)]

            onesF = const.tile([128, 1], F32, name="onesF", tag="onesF")
            nc.vector.memset(onesF, 1.0)
            onesK1 = const.tile([1, 128], F16, name="onesK1", tag="onesK1")
            nc.vector.memset(onesK1, 1.0)
            for c in range(KC):
                nc.vector.tensor_copy(
                    Vo[c][:, :, DH], onesF[:, 0:1].broadcast_to([128, NH]))

            # warm-up junk matmuls lift the PE HAM clock gate while DMAs land
            dumA = const.tile([128, 64], F16, name="dumA", tag="dumA")
            dumB = const.tile([128, 512], F16, name="dumB", tag="dumB")
            nc.vector.memset(dumA, 0.0)
            nc.vector.memset(dumB, 0.0)

            def warm_mm(n=1):
                for _ in range(n):
                    ps = psA.tile([64, 512], F32, name="warm", tag="proj")
                    nc.tensor.matmul(ps, dumA, dumB, start=True, stop=True)

            warm_mm(8)

            # ---- projection helpers ----
            def q_proj(p, sct):
                ssl = slice(sct * NS, (sct + 1) * NS)
                msl = slice(p * 128, (p + 1) * 128)
                ps = psA.tile([128, NS], F32, name="qproj", tag="proj")
                for ki in range(4):
                    nc.tensor.matmul(ps, wq[ki][:, msl], xts[ki][:, ssl],
                                     start=(ki == 0), stop=(ki == 3))
                nc.vector.tensor_copy(QT[p][:, ssl], ps)

            def k_proj(p, nt):
                nsl = slice(nt * 512, (nt + 1) * 512)
                msl = slice(p * 128, (p + 1) * 128)
                ps = psA.tile([128, 512], F32, name="kproj", tag="proj")
                for ki in range(4):
                    nc.tensor.matmul(ps, wk[ki][:, msl], ctx[ki][:, nsl],
                                     start=(ki == 0), stop=(ki == 3))
                nc.vector.tensor_copy(KT[p][:, nsl], ps)

            def v_proj(c):
                csl = slice(c * 128, (c + 1) * 128)
                ps = psA.tile([128, INNER], F32, name="vproj", tag="proj")
                for ki in range(4):
                    nc.tensor.matmul(ps, ctx[ki][:, csl], wv[ki],
                                     start=(ki == 0), stop=(ki == 3))
                nc.vector.tensor_copy(
                    Vo[c][:, :, 0:DH],
                    ps.rearrange("p (h d) -> p h d", h=NH))

            # 2-matmul staggered projection groups (slot-B in psA); state
            # dict holds the accumulating tile across the two filler slots
            fil = {}

            def kq_2mm(kind, p, nt_or_sct, half):
                if kind == "k":
                    nsl = slice(nt_or_sct * 512, (nt_or_sct + 1) * 512)
                    src = ctx
                    w = wk
                    dst, dsl = KT[p], nsl
                else:
                    nsl = slice(nt_or_sct * NS, (nt_or_sct + 1) * NS)
                    src = xts
                    w = wq
                    dst, dsl = QT[p], nsl
                msl = slice(p * 128, (p + 1) * 128)
                if half == 0:
                    fil["g"] = psA.tile([128, 512], F32, name="kq2", tag="proj")
                    for ki in (0, 1):
                        nc.tensor.matmul(fil["g"], w[ki][:, msl], src[ki][:, nsl],
                                         start=(ki == 0), stop=False)
                else:
                    for ki in (2, 3):
                        nc.tensor.matmul(fil["g"], w[ki][:, msl], src[ki][:, nsl],
                                         start=False, stop=(ki == 3))
                    nc.vector.tensor_copy(dst[:, dsl], fil["g"])

            # output staging: casts land in a packed tile per section,
            # shipped per 128-row block on alternating queues
            ysbF = [const.tile([128, 4, NS], F16, name=f"ysbF{s}", tag=f"ysbF{s}")
                    for s in range(2)]

            def out_proj(mi, sct):
                msl = slice(mi * 128, (mi + 1) * 128)
                ssl = slice(sct * NS, (sct + 1) * NS)
                ps = psA.tile([128, NS], F32, name="oproj", tag="proj")
                for ki in range(2):
                    nc.tensor.matmul(ps, wo[ki][:, msl], OT[ki][:, ssl],
                                     start=(ki == 0), stop=(ki == 1))
                nc.vector.tensor_copy(ysbF[sct][:, mi, :], ps)

            def ship_y(sct, mi):
                q = nc.sync if mi % 2 == 0 else nc.gpsimd
                q.dma_start(out=yT_d[mi * 128:(mi + 1) * 128,
                                     sct * NS:(sct + 1) * NS],
                            in_=ysbF[sct][:, mi, :])

            def norm(sct, p, o2a, o2b, last=False):
                """OT[p][:, sct] = o2 * (1/rowsum).  o2 and sums leave PSUM
                via DVE copies (freeing the banks early).  Mid-kernel the
                sums round-trip through DRAM for the [128,8] parallel
                reciprocal + partition broadcast (latency hides behind the
                exp stream); the last norm instead computes 1/s on ACT
                (idle after the final exp; one table switch) and broadcasts
                via two K=1 matmuls to avoid exposed DMA latency."""
                ssl = slice(sct * NS, (sct + 1) * NS)
                oU = nrp.tile([128, NS], F16, name="oU", tag="oU")
                ss2 = nrp.tile([1, 2 * NS], F32, name="ss2", tag="ss2")
                # sums first: the reciprocal chain is the critical path.
                # In the tail ACT is idle, so it extracts the sums itself
                # while DVE does the oU copies in parallel.
                if last:
                    nc.scalar.copy(ss2[:, 0:NS], o2a[DH:DH + 1, :])
                    nc.scalar.copy(ss2[:, NS:2 * NS], o2b[DH:DH + 1, :])
                else:
                    nc.vector.tensor_copy(ss2[:, 0:NS], o2a[DH:DH + 1, :])
                    nc.vector.tensor_copy(ss2[:, NS:2 * NS], o2b[DH:DH + 1, :])
                nc.vector.tensor_copy(oU[0:DH, :], o2a[0:DH, :])
                nc.vector.tensor_copy(oU[DH:128, :], o2b[0:DH, :])
                if last:
                    # 1/s = exp(-ln s); ACT is idle after the final exp so
                    # the table switches are cheaper than exposed DMA latency
                    lnss = nrp.tile([1, 2 * NS], F32, name="lnss", tag="lnss")
                    nc.scalar.activation(lnss, ss2, AF.Ln)
                    rr = nrp.tile([1, 2 * NS], F16, name="rr", tag="rr")
                    nc.scalar.activation(rr, lnss, AF.Exp, scale=-1.0)
                    nrmP = psA.tile([128, NS], F32, name="nrmP", tag="proj")
                    nc.tensor.matmul(nrmP[0:DH, :], onesK1[0:1, 0:DH],
                                     rr[0:1, 0:NS], start=True, stop=True)
                    nc.tensor.matmul(nrmP[DH:128, :], onesK1[0:1, DH:128],
                                     rr[0:1, NS:2 * NS], start=True, stop=True)
                    nc.vector.tensor_mul(OT[p][:, ssl], oU, nrmP)
                    return
                st = nrp.tile([128, 8], F32, name="st", tag="st")
                nc.sync.dma_start(out=st, in_=ss2)
                sr = nrp.tile([128, 8], F16, name="sr", tag="sr")
                nc.vector.reciprocal(sr, st)
                d = scr_d[2 * sct + p]
                nc.sync.dma_start(out=d[:], in_=sr)
                nrm = nrp.tile([128, NS], F16, name="nrm", tag="nrm")
                nc.sync.dma_start(
                    out=nrm,
                    in_=bass.AP(d[:].tensor, 0, [[NS, 2], [0, DH], [1, NS]]))
                # the final multiply waits on the DMA round-trip; emitting
                # it here would park it at the head of the DVE FIFO and
                # stall the next section's exp chain behind it.  Defer it
                # into the next section's fillers (the round-trip has
                # landed by then).
                fil["norm"] = (p, ssl, oU, nrm)

            # ---- minimal upfront: only what (sect0, pair0) chunk 0 needs
            # (extra V chunks ride the first two filler slots, emitted
            # after the first scores so they never delay the exp start) ----
            k_proj(0, 0)
            q_proj(0, 0)
            v_proj(0)
            v_proj(1)

            # ---- attention: 4 (section, pair) iterations of 16 chunks ----
            iters = [(0, 0), (1, 0), (0, 1), (1, 1)]

            def emit_scores(sec, c):
                sct, p = sec
                ssl = slice(sct * NS, (sct + 1) * NS)
                csl = slice(c * 128, (c + 1) * 128)
                s = psS.tile([128, 2, NS], F32, name="s", tag="s")
                nc.tensor.matmul(s[:, 0, :], KT[p][0:DH, csl], QT[p][0:DH, ssl],
                                 start=True, stop=True, tile_position=(0, 0))
                nc.tensor.matmul(s[:, 1, :], KT[p][DH:128, csl], QT[p][DH:128, ssl],
                                 start=True, stop=True, tile_position=(64, 0))
                return s

            def fillers(si, c):
                if c == 6 and "norm" in fil:
                    # finish the previous section's deferred normalization;
                    # by chunk 6 the reciprocal round-trip has landed even
                    # behind the fastest predecessor section, so this
                    # multiply never blocks the DVE FIFO
                    p_, ssl_, oU_, nrm_ = fil.pop("norm")
                    nc.vector.tensor_mul(OT[p_][:, ssl_], oU_, nrm_)
                if si == 0:
                    # V chunks stay two ahead of their o2 consumer (slot A);
                    # K0 nt1-3 + Q0 sct1 as 2-matmul groups (slot B),
                    # spread to the latest slots their consumers allow;
                    # deferred bias chunks stream on the sync/gpsimd queues
                    if c <= 1:
                        v_proj(2 * c + 2)
                        v_proj(2 * c + 3)
                    elif 4 <= c <= 13:
                        v_proj(c + 2)
                    if c <= 1:
                        kq_2mm("k", 0, 1, c)
                    elif 4 <= c <= 5:
                        kq_2mm("k", 0, 2, c - 4)
                    elif 8 <= c <= 9:
                        kq_2mm("k", 0, 3, c - 8)
                    elif 10 <= c <= 11:
                        kq_2mm("q", 0, 1, c - 10)
                elif si == 1:
                    # only what si=2's first chunks need (the rest of
                    # pair-1's K/Q projections ride in si=2), placed after
                    # c=4 so section-0 stragglers drain off the PE first
                    if 4 <= c <= 5:
                        kq_2mm("k", 1, 0, c - 4)
                    elif 8 <= c <= 9:
                        kq_2mm("q", 1, 0, c - 8)
                elif si == 2:
                    if c <= 5:
                        kq_2mm("k", 1, 1 + (c // 2), c % 2)
                    elif c <= 7:
                        kq_2mm("q", 1, 1, c % 2)
                elif si == 3:
                    # section-0 output projection, placed after the c==6
                    # deferred norm(0,1) finish and skewed late: the exp
                    # stream ends at c15, so churn near the boundary
                    # overlaps its natural tail instead of its dense middle
                    if c in (9, 11, 13, 15):
                        mi = (9, 11, 13, 15).index(c)
                        out_proj(mi, 0)
                        ship_y(0, mi)

            s_cur = emit_scores(iters[0], 0)
            for si, (sct, p) in enumerate(iters):
                o2a = psO.tile([DH + 1, NS], F32, name="o2a", tag="o2")
                o2b = psO.tile([DH + 1, NS], F32, name="o2b", tag="o2")
                for c in range(KC):
                    # scores for the next chunk go first so the PE works
                    # ahead while ACT/DVE process the current one
                    s_nxt = None
                    if c < KC - 1:
                        s_nxt = emit_scores((sct, p), c + 1)
                    elif si < 3:
                        s_nxt = emit_scores(iters[si + 1], 0)
                    e1 = work.tile([128, 2, NS], F16, name="e1", tag="e1")
                    nc.scalar.activation(e1, s_cur, AF.Exp)
                    et = work.tile([128, 2, NS], F16, name="et", tag="et")
                    nc.vector.tensor_mul(
                        et, e1, _bcast2(eB[c][:, sct * NS:(sct + 1) * NS], 2))
                    fillers(si, c)
                    nc.tensor.matmul(o2a, Vo[c][:, 2 * p, :], et[:, 0, :],
                                     start=(c == 0), stop=(c == KC - 1))
                    nc.tensor.matmul(o2b, Vo[c][:, 2 * p + 1, :], et[:, 1, :],
                                     start=(c == 0), stop=(c == KC - 1))
                    s_cur = s_nxt
                norm(sct, p, o2a, o2b, last=(si == 3))

            # ---- remaining output projection (section 1) ----
            for mi in range(4):
                out_proj(mi, 1)
                ship_y(1, mi)

    nc.compile()
    return nc


_NC_CACHE = {}


def _get_nc():
    if "nc" not in _NC_CACHE:
        _NC_CACHE["nc"] = _build_nc()
    return _NC_CACHE["nc"]


def make_in_maps(x, context, bias, Wq, Wk, Wv, Wo, bo):
    x = np.asarray(x, dtype=np.float32)
    context = np.asarray(context, dtype=np.float32)
    bias = np.asarray(bias, dtype=np.float32)
    Wq = np.asarray(Wq, dtype=np.float32)
    Wk = np.asarray(Wk, dtype=np.float32)
    Wv = np.asarray(Wv, dtype=np.float32)
    Wo = np.asarray(Wo, dtype=np.float32)

    wqT = [np.ascontiguousarray(
        (Wq[hh * INNER:(hh + 1) * INNER] * SCALE).T).astype(np.float16)
        for hh in range(2)]
    wkT = [np.ascontiguousarray(
        Wk[hh * INNER:(hh + 1) * INNER].T).astype(np.float16) for hh in range(2)]
    wvT = [np.ascontiguousarray(
        Wv[hh * INNER:(hh + 1) * INNER].T).astype(np.float16) for hh in range(2)]
    woT = [np.ascontiguousarray(
        Wo[:, hh * INNER:(hh + 1) * INNER].T).astype(np.float16) for hh in range(2)]

    xTs, ctxTs, expBs = [], [], []
    for b in range(4):
        xT = x[b].T.astype(np.float16)
        ctxT = context[b].T.astype(np.float16)
        xTs.append([np.ascontiguousarray(xT[:, s * NS:(s + 1) * NS])
                    for s in range(2)])
        ctxTs.append([np.ascontiguousarray(ctxT[:, 0:512]),
                      np.ascontiguousarray(ctxT[:, 512:2048])])
        eBT = np.exp(bias[b] - BSHIFT).T.astype(np.float16)
        expBs.append([np.ascontiguousarray(eBT[:, s * NS:(s + 1) * NS])
                      for s in range(2)])

    in_maps = []
    for core in range(8):
        b, hh = core // 2, core % 2
        m = {
            "expB0": expBs[b][0], "expB1": expBs[b][1],
            "wqT": wqT[hh], "wkT": wkT[hh], "wvT": wvT[hh], "woT": woT[hh],
        }
        for s in range(2):
            m[f"xT{s}"] = xTs[b][s]
        m["ctxT0"] = ctxTs[b][0]
        m["ctxTr"] = ctxTs[b][1]
        in_maps.append(m)
    return in_maps


def kernel(x, context, bias, Wq, Wk, Wv, Wo, bo):
    nc = _get_nc()
    in_maps = make_in_maps(x, context, bias, Wq, Wk, Wv, Wo, bo)
    res = bass_utils.run_bass_kernel_spmd(
        nc, in_maps, core_ids=list(range(8)), trace=False)

    bo = np.asarray(bo, dtype=np.float32)
    out = np.empty((4, NQ, D), dtype=np.float32)
    for b in range(4):
        yT = (res.results[2 * b]["yT"].astype(np.float32)
              + res.results[2 * b + 1]["yT"].astype(np.float32))
        out[b] = yT.T + bo
    return out
